# revision 29
# baseline (speedup 1.0000x reference)
"""Trainium2 Bass kernel for nn_EncodingModule2d (vq_codebook).

Pipeline per batch item (pure data parallel, 1 item per NeuronCore, 8 cores):
  stem:   y = relu(BN2(conv_w @ x))              -- BN folded into weights on host
  vq:     l[n,k] = s_k(|y_n|^2 - 2<y_n,c_k> + |c_k|^2)
          a = softmax_k(l)
          agg[k,:] = sum_n a[n,k] (y_n - c_k)
  post:   z = mean_k relu(BN1(agg))              -- BN folded on host
          g = sigmoid(head_w @ z + head_b)
  out:    relu(x * (1 + g))

Layout strategy (all PE operands bf16; logits/softmax/aggregate math fp32):
  - x arrives bf16 (host cast): halves input DMA vs fp32.
  - stem B: y_dn = relu(W'x + b) in (d, n) layout via 2x2 block matmuls.
  - per 128-col chunk j and c-block: ONE weight load of the y_dn chunk
    serves both a PE transpose (-> y_nd chunk, the aggregation operand) and
    a 32-col matmul against cc = -2*s_k*c[k,d] that lands the logits
    directly in (n, k) PSUM orientation - no (k, n) intermediate, no
    separate |y|^2 matmuls and no 4MB ysq tile.
  - |y_n|^2 via DVE tensor_tensor_reduce on the transposed psum (chunks 0-1)
    and a Pool square+reduce on the y_nd copy (chunks 2-3); the rank-1 term
    s_k*|y_n|^2 + s_k|c_k|^2 is composed per chunk on Pool
    (scalar_tensor_tensor) and added to the PSUM logits on DVE.
  - softmax batched per slice (4 chunks) in (n, k) layout; a stored bf16.
  - aggregation: a-chunk stationary (bf16) x [y_nd | 1] moving (257 cols),
    accumulated across all 32 chunks in one PSUM tile, interleaved into the
    main loop one slice behind softmax.
"""

import os
import sys

for _p in ("/opt/trn_rl_repo",):
    if _p not in sys.path and os.path.isdir(_p):
        sys.path.insert(0, _p)

from contextlib import ExitStack

import numpy as np
import ml_dtypes

import concourse.bass as bass
import concourse.tile as tile
from concourse import bacc, mybir
from concourse.bass_utils import run_bass_kernel_spmd
from concourse.masks import make_identity

F32 = mybir.dt.float32
BF16 = mybir.dt.bfloat16
AF = mybir.ActivationFunctionType
ALU = mybir.AluOpType
NPBF = ml_dtypes.bfloat16

B, D, H, W, K = 8, 256, 64, 64, 32
HW = H * W          # 4096 spatial positions
NB = D // 128       # 2 channel blocks of 128
NS = HW // 512      # 8 n-slices of 512
NCH = HW // 128     # 32 n-chunks of 128
CW = D + 1          # y_nd chunk width: 256 y cols + ones col
EPS = 1e-5
N_CORES = 8


def _strided_cols(t, start, step, count, width):
    """AP over columns [start + i*step : start + i*step + width) of a 2D tile."""
    a = t[:, start : start + 1]
    return bass.AP(tensor=a.tensor, offset=a.offset, ap=[a.ap[0], [step, count], [1, width]])


def _build_program():
    nc = bacc.Bacc("TRN2", target_bir_lowering=False, debug=False, num_devices=N_CORES)

    x_d = nc.dram_tensor("x", [D, HW], BF16, kind="ExternalInput").ap()
    # stem weights (early) and head weights (late), bf16
    pw_d = nc.dram_tensor("packw", [D, 256], BF16, kind="ExternalInput").ap()
    ph_d = nc.dram_tensor("packh", [D, 256], BF16, kind="ExternalInput").ap()
    # idcc: [I128 | cc_c (32)] per c-block, bf16 (fused transpose+logits rhs)
    ic_d = nc.dram_tensor("idcc", [D, 160], BF16, kind="ExternalInput").ap()
    # ssc row-broadcast consts: [scales (32) | scales*|c|^2 (32)]
    ssc_d = nc.dram_tensor("ssc", [1, 2 * K], F32, kind="ExternalInput").ap()
    sm_d = nc.dram_tensor("small", [K, D], F32, kind="ExternalInput").ap()   # centers (k,d)
    chv_d = nc.dram_tensor("chv", [D, 4], F32, kind="ExternalInput").ap()   # bias2,s1,bb1,-hb
    out_d = nc.dram_tensor("out", [D, HW], F32, kind="ExternalOutput").ap()

    with tile.TileContext(nc) as tc, ExitStack() as ctx:
        sb = ctx.enter_context(tc.tile_pool(name="sb", bufs=1))

        # ---- SBUF tiles -------------------------------------------------
        x_sb = sb.tile([128, NB, HW], BF16)
        packw = sb.tile([128, NB, 256], BF16)
        packh = sb.tile([128, NB, 256], BF16)
        idcc = sb.tile([128, NB, 160], BF16)
        srow = sb.tile([128, 2 * K], F32)        # [scales | s*|c|^2] per partition
        ckd = sb.tile([K, D], F32)
        chv = sb.tile([128, NB, 4], F32)

        y_dn = sb.tile([128, NB, HW], BF16)      # relu(W'x): d on partitions
        y_nd = sb.tile([128, NCH * CW], BF16)    # per chunk: 256 y cols + ones
        y2c = sb.tile([128, NCH], F32)           # |y_n|^2 per chunk column
        sqp = sb.tile([128, 2, 4 * D], F32)      # pool square scratch (2 slices)
        sqh = sb.tile([128, 2, 4 * 128], F32)    # folded squares (2 slices)
        y2s = sb.tile([128, 2, 4 * K], F32)      # srep*y2 per slice (rotating)
        esub = sb.tile([128, NS, 4 * K], F32)    # logits - max
        e_sb = sb.tile([128, NS, 4 * K], F32)    # exp(...)
        a_sb = sb.tile([128, NCH * K], BF16)     # softmax weights
        maxt = sb.tile([128, NCH], F32)
        sumt = sb.tile([128, NCH], F32)
        rcpt = sb.tile([128, NCH], F32)
        out_sb = sb.tile([128, NB, HW], F32)

        ident32 = sb.tile([32, 32], F32)

        # ---- DMA loads: 3 queues ---------------------------------------
        # sync: x c-block 0 pieces; scalar: x c-block 1; vector: weights
        pieces = [(0, 512), (512, 1024), (1024, 2048), (2048, 3072), (3072, 4096)]
        qeng = [nc.sync, nc.scalar]
        nc.gpsimd.dma_start(packw[:], pw_d.rearrange("(c p) m -> p c m", p=128))
        nc.gpsimd.dma_start(idcc[:], ic_d.rearrange("(c p) m -> p c m", p=128))
        nc.gpsimd.dma_start(srow[:], ssc_d.partition_broadcast(128))
        nc.gpsimd.dma_start(ckd[:], sm_d)
        nc.gpsimd.dma_start(chv[:], chv_d.rearrange("(c p) m -> p c m", p=128))
        nc.gpsimd.dma_start(packh[:], ph_d.rearrange("(c p) m -> p c m", p=128))
        for lo, hi in pieces:
            cs = slice(lo, hi)
            for c in range(NB):
                qeng[c].dma_start(x_sb[:, c, cs], x_d[c * 128 : (c + 1) * 128, cs])
        make_identity(nc, ident32[:])

        wT = packw[:, :, :]                      # (c, o) stem weights
        hwT = packh[:, :, :]                     # head_w.T / K (bf16)
        srep = srow[:, 0:K]                      # scales row (bcast over partitions)
        sc2r = srow[:, K : 2 * K]                # s_k*|c_k|^2 row

        # warm the exp table on ACT early (hidden under the x DMA)
        warm = sb.tile([128, 1], F32)
        nc.vector.memset(warm[:], 0.0)
        nc.scalar.activation(warm[:], warm[:], AF.Exp)

        # DMA-independent dummy operand for the PE HAM warm-up
        wdum = sb.tile([128, 128], BF16)
        nc.vector.memset(wdum[:], 0.5)

        # ones columns of y_nd
        nc.vector.memset(_strided_cols(y_nd, D, CW, NCH, 1), 1.0)

        with ExitStack() as stem_ctx:
            psB = stem_ctx.enter_context(tc.tile_pool(name="psB", bufs=2, space="PSUM"))
            psA = stem_ctx.enter_context(tc.tile_pool(name="psA", bufs=3, space="PSUM"))
            psL = stem_ctx.enter_context(tc.tile_pool(name="psL", bufs=2, space="PSUM"))
            psG = stem_ctx.enter_context(tc.tile_pool(name="psG", bufs=1, space="PSUM"))

            pagg = psG.tile([K, CW], F32)

            # HAM warm-up: dummy matmuls keep the PE dense until the
            # first x piece + weights arrive.
            for i in range(10):
                pW = psA.tile([128, 2, 128], BF16, name="warm", tag="pA")
                for u in range(2):
                    nc.tensor.transpose(pW[:, u, :], wdum[:], wdum[:])

            pA_tiles = {}
            pL_tiles = {}

            def emit_stem(s):
                ns = slice(s * 512, (s + 1) * 512)
                for o in range(NB):
                    pB = psB.tile([128, 512], F32, tag="pB")
                    for c in range(NB):
                        nc.tensor.matmul(
                            pB[:],
                            wT[:, c, o * 128 : (o + 1) * 128],
                            x_sb[:, c, ns],
                            start=(c == 0),
                            stop=(c == NB - 1),
                        )
                    dst = y_dn[:, o, ns]
                    if o == 0:
                        nc.scalar.activation(dst, pB[:], AF.Relu, bias=chv[:, o, 0:1])
                    else:
                        nc.vector.tensor_scalar(
                            out=dst, in0=pB[:], scalar1=chv[:, o, 0:1],
                            scalar2=0.0, op0=ALU.add, op1=ALU.max)

            def emit_tl(s):
                """Per chunk: PE transposes y chunk (bf16) into a per-chunk
                psA tile; logits pieces for the whole slice land in one
                per-slice psL tile [c-block][chunk][k] so the compose can
                batch over chunks."""
                pL = psL.tile([128, 2, 4 * K], F32, tag="pL")
                pL_tiles[s] = pL
                for jj in range(4):
                    j = 4 * s + jj
                    jc = slice(j * 128, (j + 1) * 128)
                    pA = psA.tile([128, 2, 128], BF16, tag="pA")
                    pA_tiles[(s, jj)] = pA
                    for c in range(NB):
                        nc.tensor.transpose(pA[:, c, :], y_dn[:, c, jc],
                                            idcc[:, 0, 0:128])
                        nc.tensor.matmul(pL[:, c, jj * K : (jj + 1) * K],
                                         y_dn[:, c, jc], idcc[:, c, 128:160],
                                         start=True, stop=True)

            def emit_softmax(s):
                """Off-PE per slice: y_nd copies (ACT), batched squares
                (Pool, f32 out), batched |y|^2 reduce + compose (DVE),
                esc/sub/norm on Pool, batched exp (ACT)."""
                sp = s % 2
                for jj in range(4):
                    j = 4 * s + jj
                    pA = pA_tiles.pop((s, jj))
                    dst = y_nd[:, j * CW : j * CW + D]
                    nc.scalar.activation(
                        dst.rearrange("p (u d) -> p u d", u=2), pA[:],
                        AF.Identity)
                # squares for the slice (256 y cols per chunk, ones col
                # excluded), one Pool fold halves the DVE reduce width
                base = 4 * s * CW
                ysl = bass.AP(tensor=y_nd.tensor,
                              offset=y_nd[:, base : base + 1].offset,
                              ap=[y_nd[:, 0:1].ap[0], [CW, 4], [1, D]])
                sq3 = sqp[:, sp, :].rearrange("p (g d) -> p g d", g=4)
                nc.gpsimd.tensor_mul(sq3, ysl, ysl)
                lo = bass.AP(tensor=sqp.tensor,
                             offset=sqp[:, sp, 0:1].offset,
                             ap=[sqp[:, 0, 0:1].ap[0], [D, 4], [1, 128]])
                hi = bass.AP(tensor=sqp.tensor,
                             offset=sqp[:, sp, 128:129].offset,
                             ap=[sqp[:, 0, 0:1].ap[0], [D, 4], [1, 128]])
                nc.gpsimd.tensor_tensor(
                    out=sqh[:, sp, :].rearrange("p (g d) -> p g d", g=4),
                    in0=lo, in1=hi, op=ALU.add)
                gs = slice(4 * s, 4 * s + 4)
                nc.vector.tensor_reduce(
                    out=y2c[:, gs],
                    in_=sqh[:, sp, :].rearrange("p (g d) -> p g d", g=4),
                    axis=mybir.AxisListType.X, op=ALU.add)
                pL = pL_tiles.pop(s)
                for jj in range(4):
                    j = 4 * s + jj
                    nc.vector.scalar_tensor_tensor(
                        out=y2s[:, sp, jj * K : (jj + 1) * K], in0=srep,
                        scalar=y2c[:, j : j + 1], in1=sc2r,
                        op0=ALU.mult, op1=ALU.add)
                e3 = lambda t: t[:, s, :].rearrange("p (g k) -> p g k", g=4)
                y3 = y2s[:, sp, :].rearrange("p (g k) -> p g k", g=4)
                nc.vector.tensor_tensor(out=e3(esub), in0=y3, in1=pL[:, 0, :]
                                        .rearrange("p (g k) -> p g k", g=4),
                                        op=ALU.add)
                nc.vector.tensor_tensor(out=e3(esub), in0=e3(esub),
                                        in1=pL[:, 1, :]
                                        .rearrange("p (g k) -> p g k", g=4),
                                        op=ALU.add)
                nc.vector.tensor_reduce(out=maxt[:, gs], in_=e3(esub),
                                        axis=mybir.AxisListType.X, op=ALU.max,
                                        negate=True)
                mb = maxt[:, gs].rearrange("p (g u) -> p g u", u=1).broadcast_to((128, 4, K))
                nc.gpsimd.tensor_tensor(out=e3(esub), in0=e3(esub), in1=mb,
                                        op=ALU.add)
                nc.scalar.activation(e_sb[:, s, :], esub[:, s, :], AF.Exp)
                nc.vector.tensor_reduce(out=sumt[:, gs], in_=e3(e_sb),
                                        axis=mybir.AxisListType.X, op=ALU.add)
                nc.vector.reciprocal(rcpt[:, gs], sumt[:, gs])
                rb = rcpt[:, gs].rearrange("p (g u) -> p g u", u=1).broadcast_to((128, 4, K))
                nc.gpsimd.tensor_tensor(
                    out=a_sb[:, s * 4 * K : (s + 1) * 4 * K].rearrange(
                        "p (g k) -> p g k", g=4),
                    in0=e3(e_sb), in1=rb, op=ALU.mult)

            def emit_agg(s):
                for jj in range(4):
                    g = 4 * s + jj
                    nc.tensor.matmul(
                        pagg[:],
                        a_sb[:, g * K : (g + 1) * K],
                        y_nd[:, g * CW : (g + 1) * CW],
                        start=(g == 0), stop=(g == NCH - 1),
                        skip_group_check=True)

            # ---- main pipeline ----------------------------------------
            done_tl = -1
            for s in range(NS):
                emit_stem(s)
                t = s if s <= 2 else s - 1
                if t > done_tl:
                    emit_tl(t)
                    emit_softmax(t)
                    done_tl = t
                if s == 0:
                    # hold the HAM clock warm across the x piece-1 DMA wait
                    for i in range(22):
                        pWf = psB.tile([128, 512], F32, name="wfill", tag="pB")
                        for u in range(2):
                            nc.tensor.matmul(pWf[:, u * 128 : (u + 1) * 128],
                                             wdum[:], wdum[:],
                                             start=True, stop=True)
                if s >= 3:
                    emit_agg(s - 3)
            emit_agg(NS - 3)
            emit_tl(NS - 1)
            emit_softmax(NS - 1)
            emit_agg(NS - 2)
            emit_agg(NS - 1)

            # ---- tail: rowsum fix, BN1, head, gate, output -------------
            with ExitStack() as tail_ctx:
                psT = tail_ctx.enter_context(tc.tile_pool(name="psT", bufs=2, space="PSUM"))
                psH = tail_ctx.enter_context(tc.tile_pool(name="psH", bufs=2, space="PSUM"))

                # agg[k,d] = pagg[k,d] - rowsum_a[k] * centers[k,d]
                rsc = sb.tile([K, D], F32)
                nc.vector.tensor_scalar_mul(out=rsc[:], in0=ckd[:],
                                            scalar1=pagg[:, D : D + 1])
                agg_sb = sb.tile([K, D], F32)
                nc.vector.tensor_tensor(out=agg_sb[:], in0=pagg[:, 0:D], in1=rsc[:],
                                        op=ALU.subtract)

                # BN1 + relu + mean over k -> z per d-block (bf16 for head mm)
                zbf = sb.tile([128, NB], BF16)
                t_sb = sb.tile([128, NB, K], F32)
                z_t = sb.tile([128, NB], F32)
                for b in range(NB):
                    pT = psT.tile([128, 32], F32)
                    nc.tensor.transpose(pT[:], agg_sb[:, b * 128 : (b + 1) * 128],
                                        ident32[:])
                    nc.scalar.activation(t_sb[:, b, :], pT[:], AF.Relu,
                                         bias=chv[:, b, 2:3], scale=chv[:, b, 1:2],
                                         accum_out=z_t[:, b : b + 1])
                nc.vector.tensor_copy(zbf[:], z_t[:])

                # head: gate = 1 + sigmoid(head_w @ z + head_b)
                gate = sb.tile([128, NB], F32)
                eg = sb.tile([128, NB], F32)
                for o in range(NB):
                    pH = psH.tile([128, 1], F32)
                    for c in range(NB):
                        nc.tensor.matmul(pH[:], hwT[:, c, o * 128 : (o + 1) * 128],
                                         zbf[:, c : c + 1],
                                         start=(c == 0), stop=(c == NB - 1))
                    # exp(-(v + head_b)) ; gate = 1 + 1/(1+e)
                    nc.scalar.activation(eg[:, o : o + 1], pH[:], AF.Exp,
                                         bias=chv[:, o, 3:4], scale=-1.0)
                nc.vector.tensor_scalar_add(out=eg[:], in0=eg[:], scalar1=1.0)
                nc.vector.reciprocal(gate[:], eg[:])
                nc.vector.tensor_scalar_add(out=gate[:], in0=gate[:], scalar1=1.0)

                # gating: out = relu(x32 * gate[d]); o-block 0 pieces on
            # DVE/Pool (fast on f32, before the DMAs), o-block 1 on ACT
            # (unaffected by concurrent output DMA)
            piece_eng = [
                (0, 0, "dve"), (0, 1, "dve"),
                (1, 0, "act"), (1, 1, "act"),
                (0, 2, "dve"), (0, 3, "dve"),
                (1, 2, "act"), (1, 3, "act"),
            ]
            for o, q, eng in piece_eng:
                cs = slice(q * 1024, (q + 1) * 1024)
                if eng == "act":
                    nc.scalar.activation(out_sb[:, o, cs], x_sb[:, o, cs],
                                         AF.Relu, scale=gate[:, o : o + 1])
                elif eng == "dve":
                    nc.vector.tensor_scalar(
                        out=out_sb[:, o, cs], in0=x_sb[:, o, cs],
                        scalar1=gate[:, o : o + 1], scalar2=0.0,
                        op0=ALU.mult, op1=ALU.max)
                else:
                    nc.gpsimd.tensor_scalar(
                        out=out_sb[:, o, cs], in0=x_sb[:, o, cs],
                        scalar1=gate[:, o : o + 1], scalar2=0.0,
                        op0=ALU.mult, op1=ALU.max)
                if q % 2 == 1:
                    hh = q // 2
                    cs2 = slice(hh * 2048, (hh + 1) * 2048)
                    qeng[o].dma_start(out_d[o * 128 : (o + 1) * 128, cs2],
                                      out_sb[:, o, cs2])

    nc.compile()
    return nc


_PROGRAM_CACHE = {}


def _get_program():
    if "p" not in _PROGRAM_CACHE:
        _PROGRAM_CACHE["p"] = _build_program()
    return _PROGRAM_CACHE["p"]


def _host_params(conv_w, bn2_g, bn2_b, bn2_m, bn2_v, centers, scales,
                 bn1_g, bn1_b, bn1_m, bn1_v, head_w, head_b):
    scale2 = bn2_g / np.sqrt(bn2_v + EPS)
    wT = (conv_w * scale2[:, None]).T.astype(np.float32)             # (c, o)
    bias2 = (bn2_b - bn2_m * scale2).astype(np.float32)
    cc = (-2.0 * scales[None, :] * centers.T).astype(np.float32)     # (d, k)
    c2 = (centers * centers).sum(axis=1)
    ssc = np.concatenate([scales, scales * c2]).reshape(1, 2 * K).astype(np.float32)
    s1 = bn1_g / np.sqrt(bn1_v + EPS)
    bb1 = bn1_b - bn1_m * s1
    chv = np.stack([bias2, s1.astype(np.float32), bb1.astype(np.float32),
                    (-head_b).astype(np.float32)], axis=1).astype(np.float32)
    hwT = (head_w.T / np.float32(K)).astype(np.float32)              # (c, o)
    packw = np.ascontiguousarray(wT.astype(NPBF))
    packh = np.ascontiguousarray(hwT.astype(NPBF))
    eye = np.eye(128, dtype=np.float32)
    idcc = np.concatenate(
        [np.concatenate([eye, cc[c * 128 : (c + 1) * 128]], axis=1)
         for c in range(NB)], axis=0).astype(NPBF)                   # (256, 160)
    idcc = np.ascontiguousarray(idcc)
    return packw, packh, idcc, ssc, centers.astype(np.float32), chv, bias2


def _ensure_profile_hook():
    """Register the axon NTFF profile hook if the image lacks antenv.axon_hooks."""
    import types

    if "antenv.axon_hooks" in sys.modules:
        return
    try:
        import antenv

        mod = types.ModuleType("antenv.axon_hooks")
        _hook = [None]
        mod.set_axon_ntff_profile_hook = lambda h: _hook.__setitem__(0, h)
        mod.get_axon_ntff_profile_hook = lambda: _hook[0]
        sys.modules["antenv.axon_hooks"] = mod
        antenv.axon_hooks = mod
        from trn_agent_boot.trn_boot import _ntff_profile_via_ctypes

        mod.set_axon_ntff_profile_hook(
            _ntff_profile_via_ctypes("/opt/axon/libaxon_pjrt.so"))
        import concourse.bass_utils as _bu

        _bu.upload_artifacts = lambda d: d  # no artifact store in this container
    except Exception as e:  # profiling is best-effort
        print(f"profile hook setup failed: {e}", file=sys.stderr)


def kernel(x, conv_w, bn2_g, bn2_b, bn2_m, bn2_v, centers, scales,
           bn1_g, bn1_b, bn1_m, bn1_v, head_w, head_b):
    x = np.asarray(x, dtype=np.float32)
    packw, packh, idcc, ssc, ckd, chv, bias2 = _host_params(
        np.asarray(conv_w, np.float32), np.asarray(bn2_g, np.float32),
        np.asarray(bn2_b, np.float32), np.asarray(bn2_m, np.float32),
        np.asarray(bn2_v, np.float32), np.asarray(centers, np.float32),
        np.asarray(scales, np.float32), np.asarray(bn1_g, np.float32),
        np.asarray(bn1_b, np.float32), np.asarray(bn1_m, np.float32),
        np.asarray(bn1_v, np.float32), np.asarray(head_w, np.float32),
        np.asarray(head_b, np.float32))
    nc = _get_program()

    xb = np.ascontiguousarray(x.reshape(B, D, HW).astype(NPBF))
    shared = {"packw": packw, "packh": packh, "idcc": idcc, "ssc": ssc,
              "small": ckd, "chv": chv}
    in_maps = [dict(shared, x=xb[b]) for b in range(N_CORES)]

    trace = bool(int(os.environ.get("KERNEL_TRACE", "0")))
    kwargs = {}
    if trace:
        _ensure_profile_hook()
        tdir = os.environ.get("KERNEL_TRACE_DIR")
        if tdir:
            os.makedirs(tdir, exist_ok=True)
            kwargs["tmpdir"] = tdir
    res = run_bass_kernel_spmd(nc, in_maps, list(range(N_CORES)), trace=trace, **kwargs)
    if trace:
        kernel.last_exec_time_ns = res.exec_time_ns
        kernel.last_results = res
    out = np.stack([res.results[b]["out"].reshape(D, H, W) for b in range(N_CORES)])
    return out.astype(np.float32)


# revision 33
# speedup vs baseline: 1.0518x; 1.0518x over previous
"""Trainium2 Bass kernel for nn_EncodingModule2d (vq_codebook).

Pipeline per batch item (pure data parallel, 1 item per NeuronCore, 8 cores):
  stem:   y = relu(BN2(conv_w @ x))              -- BN folded into weights on host
  vq:     l[n,k] = s_k(|y_n|^2 - 2<y_n,c_k> + |c_k|^2)
          a = softmax_k(l)
          agg[k,:] = sum_n a[n,k] (y_n - c_k)
  post:   z = mean_k relu(BN1(agg))              -- BN folded on host
          g = sigmoid(head_w @ z + head_b)
  out:    relu(x * (1 + g))

Layout strategy (all PE operands bf16; logits/softmax/aggregate math fp32):
  - x arrives bf16 (host cast): halves input DMA vs fp32.
  - stem B: y_dn = relu(W'x + b) in (d, n) layout via 2x2 block matmuls.
  - per 128-col chunk j and c-block: ONE weight load of the y_dn chunk
    serves both a PE transpose (-> y_nd chunk, the aggregation operand) and
    a 32-col matmul against cc = -2*s_k*c[k,d] that lands the logits
    directly in (n, k) PSUM orientation - no (k, n) intermediate, no
    separate |y|^2 matmuls and no 4MB ysq tile.
  - |y_n|^2 via DVE tensor_tensor_reduce on the transposed psum (chunks 0-1)
    and a Pool square+reduce on the y_nd copy (chunks 2-3); the rank-1 term
    s_k*|y_n|^2 + s_k|c_k|^2 is composed per chunk on Pool
    (scalar_tensor_tensor) and added to the PSUM logits on DVE.
  - softmax batched per slice (4 chunks) in (n, k) layout; a stored bf16.
  - aggregation: a-chunk stationary (bf16) x [y_nd | 1] moving (257 cols),
    accumulated across all 32 chunks in one PSUM tile, interleaved into the
    main loop one slice behind softmax.
"""

import os
import sys

for _p in ("/opt/trn_rl_repo",):
    if _p not in sys.path and os.path.isdir(_p):
        sys.path.insert(0, _p)

from contextlib import ExitStack

import numpy as np
import ml_dtypes

import concourse.bass as bass
import concourse.tile as tile
from concourse import bacc, mybir
from concourse.bass_utils import run_bass_kernel_spmd
from concourse.masks import make_identity

F32 = mybir.dt.float32
BF16 = mybir.dt.bfloat16
AF = mybir.ActivationFunctionType
ALU = mybir.AluOpType
NPBF = ml_dtypes.bfloat16

B, D, H, W, K = 8, 256, 64, 64, 32
HW = H * W          # 4096 spatial positions
NB = D // 128       # 2 channel blocks of 128
NS = HW // 512      # 8 n-slices of 512
NCH = HW // 128     # 32 n-chunks of 128
CW = D + 1          # y_nd chunk width: 256 y cols + ones col
EPS = 1e-5
N_CORES = 8


def _strided_cols(t, start, step, count, width):
    """AP over columns [start + i*step : start + i*step + width) of a 2D tile."""
    a = t[:, start : start + 1]
    return bass.AP(tensor=a.tensor, offset=a.offset, ap=[a.ap[0], [step, count], [1, width]])


def _build_program():
    nc = bacc.Bacc("TRN2", target_bir_lowering=False, debug=False, num_devices=N_CORES)

    x_d = nc.dram_tensor("x", [D, HW], BF16, kind="ExternalInput").ap()
    # stem weights (early) and head weights (late), bf16
    pw_d = nc.dram_tensor("packw", [D, 256], BF16, kind="ExternalInput").ap()
    ph_d = nc.dram_tensor("packh", [D, 256], BF16, kind="ExternalInput").ap()
    # idcc: [I128 | cc_c (32)] per c-block, bf16 (fused transpose+logits rhs)
    ic_d = nc.dram_tensor("idcc", [D, 160], BF16, kind="ExternalInput").ap()
    # ssc row-broadcast consts: [scales (32) | scales*|c|^2 (32)]
    ssc_d = nc.dram_tensor("ssc", [1, 2 * K], F32, kind="ExternalInput").ap()
    sm_d = nc.dram_tensor("small", [K, D], F32, kind="ExternalInput").ap()   # centers (k,d)
    chv_d = nc.dram_tensor("chv", [D, 4], F32, kind="ExternalInput").ap()   # bias2,s1,bb1,-hb
    out_d = nc.dram_tensor("out", [D, HW], F32, kind="ExternalOutput").ap()

    with tile.TileContext(nc) as tc, ExitStack() as ctx:
        sb = ctx.enter_context(tc.tile_pool(name="sb", bufs=1))

        # ---- SBUF tiles -------------------------------------------------
        x_sb = sb.tile([128, NB, HW], BF16)
        packw = sb.tile([128, NB, 256], BF16)
        packh = sb.tile([128, NB, 256], BF16)
        idcc = sb.tile([128, NB, 160], BF16)
        srow = sb.tile([128, 2 * K], F32)        # [scales | s*|c|^2] per partition
        ckd = sb.tile([K, D], F32)
        chv = sb.tile([128, NB, 4], F32)

        y_dn = sb.tile([128, NB, HW], BF16)      # relu(W'x): d on partitions
        y_nd = sb.tile([128, NCH * CW], BF16)    # per chunk: 256 y cols + ones
        y2c = sb.tile([128, NCH], F32)           # |y_n|^2 per chunk column
        sqp = sb.tile([128, 2, 4 * D], F32)      # pool square scratch (2 slices)
        sqh = sb.tile([128, 2, 4 * 128], F32)    # folded squares (2 slices)
        y2s = sb.tile([128, 2, 4 * K], F32)      # srep*y2 per slice (rotating)
        esub = sb.tile([128, NS, 4 * K], F32)    # logits - max
        e_sb = sb.tile([128, NS, 4 * K], F32)    # exp(...)
        a_sb = sb.tile([128, NCH * K], BF16)     # softmax weights
        maxt = sb.tile([128, NCH], F32)
        sumt = sb.tile([128, NCH], F32)
        rcpt = sb.tile([128, NCH], F32)
        out_sb = sb.tile([128, NB, HW], F32)

        ident32 = sb.tile([32, 32], F32)

        # ---- DMA loads: 3 queues ---------------------------------------
        # sync: x c-block 0 pieces; scalar: x c-block 1; vector: weights
        pieces = [(0, 512), (512, 1024), (1024, 2048), (2048, 3072), (3072, 4096)]
        qeng = [nc.sync, nc.scalar]
        nc.gpsimd.dma_start(packw[:], pw_d.rearrange("(c p) m -> p c m", p=128))
        nc.gpsimd.dma_start(idcc[:], ic_d.rearrange("(c p) m -> p c m", p=128))
        nc.gpsimd.dma_start(srow[:], ssc_d.partition_broadcast(128))
        nc.gpsimd.dma_start(ckd[:], sm_d)
        nc.gpsimd.dma_start(chv[:], chv_d.rearrange("(c p) m -> p c m", p=128))
        nc.gpsimd.dma_start(packh[:], ph_d.rearrange("(c p) m -> p c m", p=128))
        for lo, hi in pieces:
            cs = slice(lo, hi)
            for c in range(NB):
                qeng[c].dma_start(x_sb[:, c, cs], x_d[c * 128 : (c + 1) * 128, cs])
        make_identity(nc, ident32[:])

        wT = packw[:, :, :]                      # (c, o) stem weights
        hwT = packh[:, :, :]                     # head_w.T / K (bf16)
        srep = srow[:, 0:K]                      # scales row (bcast over partitions)
        sc2r = srow[:, K : 2 * K]                # s_k*|c_k|^2 row

        # warm the exp table on ACT early (hidden under the x DMA)
        warm = sb.tile([128, 1], F32)
        nc.vector.memset(warm[:], 0.0)
        nc.scalar.activation(warm[:], warm[:], AF.Exp)

        # DMA-independent dummy operand for the PE HAM warm-up
        wdum = sb.tile([128, 128], BF16)
        nc.vector.memset(wdum[:], 0.5)

        # ones columns of y_nd
        nc.vector.memset(_strided_cols(y_nd, D, CW, NCH, 1), 1.0)

        with ExitStack() as stem_ctx:
            psB = stem_ctx.enter_context(tc.tile_pool(name="psB", bufs=2, space="PSUM"))
            psA = stem_ctx.enter_context(tc.tile_pool(name="psA", bufs=3, space="PSUM"))
            psL = stem_ctx.enter_context(tc.tile_pool(name="psL", bufs=2, space="PSUM"))
            psG = stem_ctx.enter_context(tc.tile_pool(name="psG", bufs=1, space="PSUM"))

            pagg = psG.tile([K, CW], F32)

            # HAM warm-up: dummy matmuls keep the PE dense until the
            # first x piece + weights arrive.
            for i in range(10):
                pW = psA.tile([128, 2, 128], BF16, name="warm", tag="pA")
                for u in range(2):
                    nc.tensor.transpose(pW[:, u, :], wdum[:], wdum[:])

            pA_tiles = {}
            pL_tiles = {}

            def emit_stem(s):
                ns = slice(s * 512, (s + 1) * 512)
                for o in range(NB):
                    pB = psB.tile([128, 512], F32, tag="pB")
                    for c in range(NB):
                        nc.tensor.matmul(
                            pB[:],
                            wT[:, c, o * 128 : (o + 1) * 128],
                            x_sb[:, c, ns],
                            start=(c == 0),
                            stop=(c == NB - 1),
                        )
                    dst = y_dn[:, o, ns]
                    if o == 0:
                        nc.scalar.activation(dst, pB[:], AF.Relu, bias=chv[:, o, 0:1])
                    else:
                        nc.vector.tensor_scalar(
                            out=dst, in0=pB[:], scalar1=chv[:, o, 0:1],
                            scalar2=0.0, op0=ALU.add, op1=ALU.max)

            def emit_tl(s):
                """Per chunk: PE transposes y chunk (bf16) into a per-chunk
                psA tile; logits pieces for the whole slice land in one
                per-slice psL tile [c-block][chunk][k] so the compose can
                batch over chunks."""
                pL = psL.tile([128, 2, 4 * K], F32, tag="pL")
                pL_tiles[s] = pL
                for jj in range(4):
                    j = 4 * s + jj
                    jc = slice(j * 128, (j + 1) * 128)
                    pA = psA.tile([128, 2, 128], BF16, tag="pA")
                    pA_tiles[(s, jj)] = pA
                    for c in range(NB):
                        nc.tensor.transpose(pA[:, c, :], y_dn[:, c, jc],
                                            idcc[:, 0, 0:128])
                        nc.tensor.matmul(pL[:, c, jj * K : (jj + 1) * K],
                                         y_dn[:, c, jc], idcc[:, c, 128:160],
                                         start=True, stop=True)

            def emit_softmax(s):
                """Off-PE per slice: y_nd copies (ACT), batched squares
                (Pool, f32 out), batched |y|^2 reduce + compose (DVE),
                esc/sub/norm on Pool, batched exp (ACT)."""
                sp = s % 2
                for jj in range(4):
                    j = 4 * s + jj
                    pA = pA_tiles.pop((s, jj))
                    dst = y_nd[:, j * CW : j * CW + D]
                    nc.scalar.activation(
                        dst.rearrange("p (u d) -> p u d", u=2), pA[:],
                        AF.Identity)
                # squares for the slice (256 y cols per chunk, ones col
                # excluded), one Pool fold halves the DVE reduce width
                base = 4 * s * CW
                ysl = bass.AP(tensor=y_nd.tensor,
                              offset=y_nd[:, base : base + 1].offset,
                              ap=[y_nd[:, 0:1].ap[0], [CW, 4], [1, D]])
                sq3 = sqp[:, sp, :].rearrange("p (g d) -> p g d", g=4)
                nc.gpsimd.tensor_mul(sq3, ysl, ysl)
                lo = bass.AP(tensor=sqp.tensor,
                             offset=sqp[:, sp, 0:1].offset,
                             ap=[sqp[:, 0, 0:1].ap[0], [D, 4], [1, 128]])
                hi = bass.AP(tensor=sqp.tensor,
                             offset=sqp[:, sp, 128:129].offset,
                             ap=[sqp[:, 0, 0:1].ap[0], [D, 4], [1, 128]])
                nc.gpsimd.tensor_tensor(
                    out=sqh[:, sp, :].rearrange("p (g d) -> p g d", g=4),
                    in0=lo, in1=hi, op=ALU.add)
                gs = slice(4 * s, 4 * s + 4)
                nc.vector.tensor_reduce(
                    out=y2c[:, gs],
                    in_=sqh[:, sp, :].rearrange("p (g d) -> p g d", g=4),
                    axis=mybir.AxisListType.X, op=ALU.add)
                pL = pL_tiles.pop(s)
                for jj in range(4):
                    j = 4 * s + jj
                    nc.vector.scalar_tensor_tensor(
                        out=y2s[:, sp, jj * K : (jj + 1) * K], in0=srep,
                        scalar=y2c[:, j : j + 1], in1=sc2r,
                        op0=ALU.mult, op1=ALU.add)
                e3 = lambda t: t[:, s, :].rearrange("p (g k) -> p g k", g=4)
                y3 = y2s[:, sp, :].rearrange("p (g k) -> p g k", g=4)
                nc.vector.tensor_tensor(out=e3(esub), in0=y3, in1=pL[:, 0, :]
                                        .rearrange("p (g k) -> p g k", g=4),
                                        op=ALU.add)
                nc.vector.tensor_tensor(out=e3(esub), in0=e3(esub),
                                        in1=pL[:, 1, :]
                                        .rearrange("p (g k) -> p g k", g=4),
                                        op=ALU.add)
                nc.vector.tensor_reduce(out=maxt[:, gs], in_=e3(esub),
                                        axis=mybir.AxisListType.X, op=ALU.max,
                                        negate=True)
                mb = maxt[:, gs].rearrange("p (g u) -> p g u", u=1).broadcast_to((128, 4, K))
                nc.gpsimd.tensor_tensor(out=e3(esub), in0=e3(esub), in1=mb,
                                        op=ALU.add)
                nc.scalar.activation(e_sb[:, s, :], esub[:, s, :], AF.Exp)
                nc.vector.tensor_reduce(out=sumt[:, gs], in_=e3(e_sb),
                                        axis=mybir.AxisListType.X, op=ALU.add)
                nc.vector.reciprocal(rcpt[:, gs], sumt[:, gs])
                rb = rcpt[:, gs].rearrange("p (g u) -> p g u", u=1).broadcast_to((128, 4, K))
                nc.gpsimd.tensor_tensor(
                    out=a_sb[:, s * 4 * K : (s + 1) * 4 * K].rearrange(
                        "p (g k) -> p g k", g=4),
                    in0=e3(e_sb), in1=rb, op=ALU.mult)

            def emit_agg(s, jlo=0, jn=4):
                for jj in range(jlo, jlo + jn):
                    g = 4 * s + jj
                    nc.tensor.matmul(
                        pagg[:],
                        a_sb[:, g * K : (g + 1) * K],
                        y_nd[:, g * CW : (g + 1) * CW],
                        start=(g == 0), stop=(g == NCH - 1),
                        skip_group_check=True)

            # ---- main pipeline ----------------------------------------
            done_tl = -1
            for s in range(NS):
                emit_stem(s)
                t = s if s <= 2 else s - 1
                if t > done_tl:
                    emit_tl(t)
                    emit_softmax(t)
                    done_tl = t
                if s == 0:
                    # hold the HAM clock warm across the x piece-1 DMA wait
                    for i in range(22):
                        pWf = psB.tile([128, 512], F32, name="wfill", tag="pB")
                        for u in range(2):
                            nc.tensor.matmul(pWf[:, u * 128 : (u + 1) * 128],
                                             wdum[:], wdum[:],
                                             start=True, stop=True)
                if s >= 3:
                    emit_agg(s - 3)
            emit_agg(NS - 3)
            emit_tl(NS - 1)
            emit_softmax(NS - 1, 0, 2)
            emit_agg(NS - 2)
            emit_softmax(NS - 1, 2, 2)
            emit_agg(NS - 1, 0, 2)
            emit_agg(NS - 1, 2, 2)

            # ---- tail: rowsum fix, BN1, head, gate, output -------------
            with ExitStack() as tail_ctx:
                psT = tail_ctx.enter_context(tc.tile_pool(name="psT", bufs=2, space="PSUM"))
                psH = tail_ctx.enter_context(tc.tile_pool(name="psH", bufs=2, space="PSUM"))

                # agg[k,d] = pagg[k,d] - rowsum_a[k] * centers[k,d]
                rsc = sb.tile([K, D], F32)
                nc.vector.tensor_scalar_mul(out=rsc[:], in0=ckd[:],
                                            scalar1=pagg[:, D : D + 1])
                agg_sb = sb.tile([K, D], F32)
                nc.vector.tensor_tensor(out=agg_sb[:], in0=pagg[:, 0:D], in1=rsc[:],
                                        op=ALU.subtract)

                # BN1 + relu + mean over k -> z per d-block (bf16 for head mm)
                zbf = sb.tile([128, NB], BF16)
                t_sb = sb.tile([128, NB, K], F32)
                z_t = sb.tile([128, NB], F32)
                for b in range(NB):
                    pT = psT.tile([128, 32], F32)
                    nc.tensor.transpose(pT[:], agg_sb[:, b * 128 : (b + 1) * 128],
                                        ident32[:])
                    nc.scalar.activation(t_sb[:, b, :], pT[:], AF.Relu,
                                         bias=chv[:, b, 2:3], scale=chv[:, b, 1:2],
                                         accum_out=z_t[:, b : b + 1])
                nc.vector.tensor_copy(zbf[:], z_t[:])

                # head: gate = 1 + sigmoid(head_w @ z + head_b)
                gate = sb.tile([128, NB], F32)
                eg = sb.tile([128, NB], F32)
                for o in range(NB):
                    pH = psH.tile([128, 1], F32)
                    for c in range(NB):
                        nc.tensor.matmul(pH[:], hwT[:, c, o * 128 : (o + 1) * 128],
                                         zbf[:, c : c + 1],
                                         start=(c == 0), stop=(c == NB - 1))
                    # exp(-(v + head_b)) ; gate = 1 + 1/(1+e)
                    nc.scalar.activation(eg[:, o : o + 1], pH[:], AF.Exp,
                                         bias=chv[:, o, 3:4], scale=-1.0)
                nc.vector.tensor_scalar_add(out=eg[:], in0=eg[:], scalar1=1.0)
                nc.vector.reciprocal(gate[:], eg[:])
                nc.vector.tensor_scalar_add(out=gate[:], in0=gate[:], scalar1=1.0)

                # gating: out = relu(x32 * gate[d]); o-block 0 pieces on
            # DVE/Pool (fast on f32, before the DMAs), o-block 1 on ACT
            # (unaffected by concurrent output DMA)
            piece_eng = [
                (0, 0, "dve"), (0, 1, "dve"),
                (1, 0, "act"), (1, 1, "act"),
                (0, 2, "dve"), (0, 3, "dve"),
                (1, 2, "act"), (1, 3, "act"),
            ]
            for o, q, eng in piece_eng:
                cs = slice(q * 1024, (q + 1) * 1024)
                if eng == "act":
                    nc.scalar.activation(out_sb[:, o, cs], x_sb[:, o, cs],
                                         AF.Relu, scale=gate[:, o : o + 1])
                elif eng == "dve":
                    nc.vector.tensor_scalar(
                        out=out_sb[:, o, cs], in0=x_sb[:, o, cs],
                        scalar1=gate[:, o : o + 1], scalar2=0.0,
                        op0=ALU.mult, op1=ALU.max)
                else:
                    nc.gpsimd.tensor_scalar(
                        out=out_sb[:, o, cs], in0=x_sb[:, o, cs],
                        scalar1=gate[:, o : o + 1], scalar2=0.0,
                        op0=ALU.mult, op1=ALU.max)
                if q % 2 == 1:
                    hh = q // 2
                    cs2 = slice(hh * 2048, (hh + 1) * 2048)
                    qeng[o].dma_start(out_d[o * 128 : (o + 1) * 128, cs2],
                                      out_sb[:, o, cs2])

    nc.compile()
    return nc


_PROGRAM_CACHE = {}


def _get_program():
    if "p" not in _PROGRAM_CACHE:
        _PROGRAM_CACHE["p"] = _build_program()
    return _PROGRAM_CACHE["p"]


def _host_params(conv_w, bn2_g, bn2_b, bn2_m, bn2_v, centers, scales,
                 bn1_g, bn1_b, bn1_m, bn1_v, head_w, head_b):
    scale2 = bn2_g / np.sqrt(bn2_v + EPS)
    wT = (conv_w * scale2[:, None]).T.astype(np.float32)             # (c, o)
    bias2 = (bn2_b - bn2_m * scale2).astype(np.float32)
    cc = (-2.0 * scales[None, :] * centers.T).astype(np.float32)     # (d, k)
    c2 = (centers * centers).sum(axis=1)
    ssc = np.concatenate([scales, scales * c2]).reshape(1, 2 * K).astype(np.float32)
    s1 = bn1_g / np.sqrt(bn1_v + EPS)
    bb1 = bn1_b - bn1_m * s1
    chv = np.stack([bias2, s1.astype(np.float32), bb1.astype(np.float32),
                    (-head_b).astype(np.float32)], axis=1).astype(np.float32)
    hwT = (head_w.T / np.float32(K)).astype(np.float32)              # (c, o)
    packw = np.ascontiguousarray(wT.astype(NPBF))
    packh = np.ascontiguousarray(hwT.astype(NPBF))
    eye = np.eye(128, dtype=np.float32)
    idcc = np.concatenate(
        [np.concatenate([eye, cc[c * 128 : (c + 1) * 128]], axis=1)
         for c in range(NB)], axis=0).astype(NPBF)                   # (256, 160)
    idcc = np.ascontiguousarray(idcc)
    return packw, packh, idcc, ssc, centers.astype(np.float32), chv, bias2


def _ensure_profile_hook():
    """Register the axon NTFF profile hook if the image lacks antenv.axon_hooks."""
    import types

    if "antenv.axon_hooks" in sys.modules:
        return
    try:
        import antenv

        mod = types.ModuleType("antenv.axon_hooks")
        _hook = [None]
        mod.set_axon_ntff_profile_hook = lambda h: _hook.__setitem__(0, h)
        mod.get_axon_ntff_profile_hook = lambda: _hook[0]
        sys.modules["antenv.axon_hooks"] = mod
        antenv.axon_hooks = mod
        from trn_agent_boot.trn_boot import _ntff_profile_via_ctypes

        mod.set_axon_ntff_profile_hook(
            _ntff_profile_via_ctypes("/opt/axon/libaxon_pjrt.so"))
        import concourse.bass_utils as _bu

        _bu.upload_artifacts = lambda d: d  # no artifact store in this container
    except Exception as e:  # profiling is best-effort
        print(f"profile hook setup failed: {e}", file=sys.stderr)


def kernel(x, conv_w, bn2_g, bn2_b, bn2_m, bn2_v, centers, scales,
           bn1_g, bn1_b, bn1_m, bn1_v, head_w, head_b):
    x = np.asarray(x, dtype=np.float32)
    packw, packh, idcc, ssc, ckd, chv, bias2 = _host_params(
        np.asarray(conv_w, np.float32), np.asarray(bn2_g, np.float32),
        np.asarray(bn2_b, np.float32), np.asarray(bn2_m, np.float32),
        np.asarray(bn2_v, np.float32), np.asarray(centers, np.float32),
        np.asarray(scales, np.float32), np.asarray(bn1_g, np.float32),
        np.asarray(bn1_b, np.float32), np.asarray(bn1_m, np.float32),
        np.asarray(bn1_v, np.float32), np.asarray(head_w, np.float32),
        np.asarray(head_b, np.float32))
    nc = _get_program()

    xb = np.ascontiguousarray(x.reshape(B, D, HW).astype(NPBF))
    shared = {"packw": packw, "packh": packh, "idcc": idcc, "ssc": ssc,
              "small": ckd, "chv": chv}
    in_maps = [dict(shared, x=xb[b]) for b in range(N_CORES)]

    trace = bool(int(os.environ.get("KERNEL_TRACE", "0")))
    kwargs = {}
    if trace:
        _ensure_profile_hook()
        tdir = os.environ.get("KERNEL_TRACE_DIR")
        if tdir:
            os.makedirs(tdir, exist_ok=True)
            kwargs["tmpdir"] = tdir
    res = run_bass_kernel_spmd(nc, in_maps, list(range(N_CORES)), trace=trace, **kwargs)
    if trace:
        kernel.last_exec_time_ns = res.exec_time_ns
        kernel.last_results = res
    out = np.stack([res.results[b]["out"].reshape(D, H, W) for b in range(N_CORES)])
    return out.astype(np.float32)
